# revision 1
# baseline (speedup 1.0000x reference)
"""DGCNN (3x DynamicEdgeConv + global max pool + MLP head) on 8 Trainium2
NeuronCores, data-parallel over the batch (one point cloud per core).

EdgeConv algebra: h_ij = [x_i, x_j - x_i] @ W + b = u_i + v_j with
  u = x @ (Wa - Wb) + b,  v = x @ Wb;  out_i = u_i + max_{j in knn(i)} v_j.
kNN key: d''_ij = 2 x_i.x_j - |x_j|^2 (nearest = largest, self = row max).
Top-24 per row via 3x(max8 + match_replace); slots 1..20 are the k=20 nearest.
Neighbor rows of v are gathered from DRAM via per-partition indirect DMA
(one offset per partition, 128 rows per call) and max-reduced on DVE.
"""
import numpy as np

_NC_CACHE = {}


def _builder(debug=False):
    import concourse.bacc as bacc
    import concourse.mybir as mybir
    from concourse.bass import IndirectOffsetOnAxis
    from concourse.tile import TileContext

    F32 = mybir.dt.float32
    U32 = mybir.dt.uint32
    AF = mybir.ActivationFunctionType
    ALU = mybir.AluOpType
    AX = mybir.AxisListType
    N, NT, JC = 2048, 16, 4

    def ts(i, s):
        return slice(i * s, (i + 1) * s)

    nc = bacc.Bacc("TRN2", num_devices=8)

    xT = nc.dram_tensor("xT", [3, N], F32, kind="ExternalInput").ap()
    idn_in = nc.dram_tensor("idn", [128, 128], F32, kind="ExternalInput").ap()
    nsq1_in = nc.dram_tensor("nsq1", [1, N], F32, kind="ExternalInput").ap()
    AB1 = nc.dram_tensor("AB1", [4, 64], F32, kind="ExternalInput").ap()
    BB1 = nc.dram_tensor("BB1", [3, 64], F32, kind="ExternalInput").ap()
    AB2 = nc.dram_tensor("AB2", [65, 128], F32, kind="ExternalInput").ap()
    BB2 = nc.dram_tensor("BB2", [64, 128], F32, kind="ExternalInput").ap()
    AB3 = nc.dram_tensor("AB3", [128, 256], F32, kind="ExternalInput").ap()
    BB3 = nc.dram_tensor("BB3", [128, 256], F32, kind="ExternalInput").ap()
    b3r = nc.dram_tensor("b3r", [1, 256], F32, kind="ExternalInput").ap()
    fc1w = nc.dram_tensor("fc1w", [256, 512], F32, kind="ExternalInput").ap()
    fc1b = nc.dram_tensor("fc1b", [128, 4], F32, kind="ExternalInput").ap()
    fc2w = nc.dram_tensor("fc2w", [512, 256], F32, kind="ExternalInput").ap()
    fc2b = nc.dram_tensor("fc2b", [128, 2], F32, kind="ExternalInput").ap()
    fc3w = nc.dram_tensor("fc3w", [256, 16], F32, kind="ExternalInput").ap()
    fc3b = nc.dram_tensor("fc3b", [16, 1], F32, kind="ExternalInput").ap()
    out = nc.dram_tensor("out", [16, 1], F32, kind="ExternalOutput").ap()
    dbg = {}
    if debug:
        for name, shape, dt in [("h1", [128, 16, 64], F32), ("h2", [128, 16, 128], F32),
                                ("h3d", [128, 16, 256], F32), ("d0", [128, N], F32)]:
            dbg[name] = nc.dram_tensor(name, shape, dt, kind="ExternalOutput").ap()

    v_drams = {}

    def run_layer(tc, layer, C, D, hT, nsq, Acat, B, brow, ones1, out_h):
        """hT: [C(+1), N] features^T (+ ones row when C < 128); nsq: [1,N] = -|x|^2.
        Returns h in natural P16 layout [128, 16, D] (i = g*128 + p)."""
        u_aug = C + 1 <= 128
        with tc.tile_pool(name=f"L{layer}", bufs=1) as lp:
            hT2x = lp.tile([C, N], F32, name=f"hT2x_{layer}")
            nc.scalar.activation(hT2x[:], hT[0:C, :], AF.Copy, scale=2.0)

            # ---------- u/v matmuls + v -> DRAM ----------
            u = lp.tile([128, NT, D], F32, name=f"u_{layer}")
            vslice = v_drams[D]
            with tc.tile_pool(name=f"L{layer}uv", bufs=4, space="PSUM") as uvps, \
                 tc.tile_pool(name=f"L{layer}uvsb", bufs=4) as uvsb:
                for t in range(NT):
                    vp = uvps.tile([128, D], F32, name="vp")
                    nc.tensor.matmul(vp[:], hT[0:C, ts(t, 128)], B[:], start=True, stop=True)
                    up = uvps.tile([128, D], F32, name="up")
                    if u_aug:
                        nc.tensor.matmul(up[:], hT[0:C + 1, ts(t, 128)], Acat[:],
                                         start=True, stop=True)
                    else:
                        nc.tensor.matmul(up[:], hT[0:C, ts(t, 128)], Acat[:],
                                         start=True, stop=False)
                        nc.tensor.matmul(up[:], ones1[:], brow[:], start=False, stop=True)
                    vsb = uvsb.tile([128, D], F32, name="vsb")
                    nc.scalar.copy(vsb[:], vp[:])
                    nc.scalar.copy(u[:, t, :], up[:])
                    nc.sync.dma_start(vslice[ts(t, 128), :], vsb[:])

            # ---------- per-tile: dist matmuls + top-24 selection + gather ----------
            h = out_h if out_h is not None else lp.tile([128, NT, D], F32, name=f"h_{layer}")
            with tc.tile_pool(name=f"L{layer}d", bufs=2, space="PSUM") as dps, \
                 tc.tile_pool(name=f"L{layer}dsb", bufs=3) as dsb, \
                 tc.tile_pool(name=f"L{layer}sel", bufs=3) as selp, \
                 tc.tile_pool(name=f"L{layer}g", bufs=3) as gp:
                for t in range(NT):
                    dp = dps.tile([128, N], F32, name="dp")
                    for j in range(JC):
                        nc.tensor.matmul(dp[:, ts(j, 512)], hT2x[:, ts(t, 128)],
                                         hT[0:C, ts(j, 512)], start=True, stop=False)
                        nc.tensor.matmul(dp[:, ts(j, 512)], ones1[:],
                                         nsq[:, ts(j, 512)], start=False, stop=True)
                    dd = dsb.tile([128, N], F32, name="dd")
                    nc.scalar.copy(dd[:], dp[:])
                    if dbg and layer == 1 and t == 0:
                        nc.sync.dma_start(dbg["d0"], dd[:])

                    m1 = selp.tile([128, 8], F32, name="m1")
                    m2 = selp.tile([128, 8], F32, name="m2")
                    m3 = selp.tile([128, 8], F32, name="m3")
                    ww = selp.tile([128, N], F32, name="ww")
                    jtab = selp.tile([128, 24], U32, name="jtab")
                    gb = gp.tile([128, 20, D], F32, name="gb")

                    def fire(lo, hi):
                        for m in range(lo, hi):
                            nc.gpsimd.indirect_dma_start(
                                out=gb[:, m - 1, :], out_offset=None, in_=vslice,
                                in_offset=IndirectOffsetOnAxis(
                                    ap=jtab[:, m:m + 1], axis=0))

                    nc.vector.max(out=m1[:], in_=dd[:])
                    nc.vector.max_index(out=jtab[:, 0:8], in_max=m1[:], in_values=dd[:])
                    nc.vector.match_replace(out=ww[:], in_to_replace=m1[:],
                                            in_values=dd[:], imm_value=-1e30)
                    fire(1, 8)
                    nc.vector.max(out=m2[:], in_=ww[:])
                    nc.vector.max_index(out=jtab[:, 8:16], in_max=m2[:], in_values=dd[:])
                    nc.vector.match_replace(out=ww[:], in_to_replace=m2[:],
                                            in_values=ww[:], imm_value=-1e30)
                    fire(8, 16)
                    nc.vector.max(out=m3[:], in_=ww[:])
                    nc.vector.max_index(out=jtab[:, 16:24], in_max=m3[:], in_values=dd[:])
                    fire(16, 21)
                    vm = gp.tile([128, D], F32, name="vm")
                    nc.vector.tensor_reduce(out=vm[:], in_=gb.rearrange("p m d -> p d m"), axis=AX.X, op=ALU.max)
                    nc.vector.tensor_tensor(out=h[:, t, :], in0=u[:, t, :], in1=vm[:],
                                            op=ALU.add)

            if dbg and layer in (1, 2):
                nc.sync.dma_start(dbg[f"h{layer}"], h[:])
        return h

    def transpose_prep(tc, layer, h, hT_next, nsq_next, next_C, idn):
        with tc.tile_pool(name=f"L{layer}t", bufs=3, space="PSUM") as tps, \
             tc.tile_pool(name=f"L{layer}tsb", bufs=1) as tsbp:
            for t in range(NT):
                tp = tps.tile([next_C, 128], F32, name="tp")
                nc.tensor.transpose(tp[:], h[:, t, 0:next_C], idn[:])
                nc.scalar.copy(hT_next[0:next_C, ts(t, 128)], tp[:])
            if next_C + 1 <= hT_next.shape[0]:
                nc.vector.memset(hT_next[next_C:next_C + 1, :], 1.0)
            xsq = tsbp.tile([next_C, 2048], F32, name="xsq")
            nc.scalar.square(xsq[:], hT_next[0:next_C, :])
            onesC = tsbp.tile([next_C, 1], F32, name="onesC")
            nc.vector.memset(onesC[:], 1.0)
            for j in range(JC):
                sqp = tps.tile([1, 512], F32, name="sqp")
                nc.tensor.matmul(sqp[:], onesC[:], xsq[:, ts(j, 512)], start=True, stop=True)
                nc.scalar.activation(nsq_next[0:1, ts(j, 512)], sqp[:], AF.Copy, scale=-1.0)

    with TileContext(nc) as tc:
        with tc.tile_pool(name="const", bufs=1) as cp, \
             tc.tile_pool(name="feat", bufs=1) as fp, \
             tc.tile_pool(name="vdram", bufs=1, space="DRAM") as vdp:
            for _D in (64, 128, 256):
                v_drams[_D] = vdp.tile([N, _D], F32, name=f"v_dram{_D}")
            idn = cp.tile([128, 128], F32)
            nc.sync.dma_start(idn[:], idn_in)
            ones1 = cp.tile([1, 128], F32)
            nc.vector.memset(ones1[:], 1.0)
            with tc.tile_pool(name="warm", bufs=1, space="PSUM") as wps:
                wt = wps.tile([128, 128], F32)
                nc.tensor.transpose(wt[:], idn[:], idn[:])

            hT1 = fp.tile([4, N], F32)
            nsq1 = fp.tile([1, N], F32)
            hT2 = fp.tile([65, N], F32)
            nsq2 = fp.tile([1, N], F32)
            hT3 = fp.tile([128, N], F32)
            nsq3 = fp.tile([1, N], F32)
            h3 = fp.tile([128, 16, 256], F32)

            nc.vector.memset(hT1[:], 1.0)
            nc.sync.dma_start(hT1[0:3, :], xT)
            nc.sync.dma_start(nsq1[:], nsq1_in)

            with tc.tile_pool(name="wts", bufs=1) as wp:
                w = {}
                for nm, ap_, shape in [("AB1", AB1, [4, 64]), ("BB1", BB1, [3, 64]),
                                       ("AB2", AB2, [65, 128]), ("BB2", BB2, [64, 128]),
                                       ("AB3", AB3, [128, 256]), ("BB3", BB3, [128, 256]),
                                       ("b3r", b3r, [1, 256])]:
                    t = wp.tile(shape, F32, name=f"w_{nm}")
                    nc.sync.dma_start(t[:], ap_)
                    w[nm] = t

                h1 = run_layer(tc, 1, 3, 64, hT1, nsq1, w["AB1"], w["BB1"], None, ones1, None)
                transpose_prep(tc, 1, h1, hT2, nsq2, 64, idn)
                h2 = run_layer(tc, 2, 64, 128, hT2, nsq2, w["AB2"], w["BB2"], None, ones1, None)
                transpose_prep(tc, 2, h2, hT3, nsq3, 128, idn)
                run_layer(tc, 3, 128, 256, hT3, nsq3, w["AB3"], w["BB3"], w["b3r"], ones1, h3)

            with tc.tile_pool(name="head", bufs=1) as hp, \
                 tc.tile_pool(name="headps", bufs=1, space="PSUM") as hps:
                gmax = hp.tile([128, 256], F32)
                nc.vector.tensor_reduce(out=gmax[:],
                                        in_=h3.rearrange("p g d -> p d g"),
                                        axis=AX.X, op=ALU.max)
                g0 = hp.tile([128, 1], F32)
                g1 = hp.tile([128, 1], F32)
                for half, gdst in ((0, g0), (1, g1)):
                    tp = hps.tile([128, 128], F32, name="tp", tag="tp")
                    nc.tensor.transpose(tp[:], gmax[:, ts(half, 128)], idn[:])
                    tsb = hp.tile([128, 128], F32, name=f"tsb_{half}")
                    nc.scalar.copy(tsb[:], tp[:])
                    nc.vector.tensor_reduce(out=gdst[:], in_=tsb[:], axis=AX.X, op=ALU.max)

                fw1 = [hp.tile([128, 512], F32, name=f"fw1_{kk}") for kk in range(2)]
                fw2 = [hp.tile([128, 256], F32, name=f"fw2_{kk}") for kk in range(4)]
                fw3 = [hp.tile([128, 16], F32, name=f"fw3_{kk}") for kk in range(2)]
                fb1 = hp.tile([128, 4], F32)
                fb2 = hp.tile([128, 2], F32)
                fb3 = hp.tile([16, 1], F32)
                for kk in range(2):
                    nc.sync.dma_start(fw1[kk][:], fc1w[ts(kk, 128), :])
                    nc.sync.dma_start(fw3[kk][:], fc3w[ts(kk, 128), :])
                for kk in range(4):
                    nc.sync.dma_start(fw2[kk][:], fc2w[ts(kk, 128), :])
                nc.sync.dma_start(fb1[:], fc1b)
                nc.sync.dma_start(fb2[:], fc2b)
                nc.sync.dma_start(fb3[:], fc3b)

                a1 = [hp.tile([128, 1], F32, name=f"a1_{m}") for m in range(4)]
                for m in range(4):
                    p = hps.tile([128, 1], F32, name="fcp", tag="fcp")
                    nc.tensor.matmul(p[:], fw1[0][:, ts(m, 128)], g0[:], start=True, stop=False)
                    nc.tensor.matmul(p[:], fw1[1][:, ts(m, 128)], g1[:], start=False, stop=True)
                    nc.scalar.activation(a1[m][:], p[:], AF.Relu, bias=fb1[:, m:m + 1], scale=1.0)
                a2 = [hp.tile([128, 1], F32, name=f"a2_{m}") for m in range(2)]
                for m in range(2):
                    p = hps.tile([128, 1], F32, name="fcp", tag="fcp")
                    for kk in range(4):
                        nc.tensor.matmul(p[:], fw2[kk][:, ts(m, 128)], a1[kk][:],
                                         start=(kk == 0), stop=(kk == 3))
                    nc.scalar.activation(a2[m][:], p[:], AF.Relu, bias=fb2[:, m:m + 1], scale=1.0)
                p3 = hps.tile([128, 1], F32, name="fcp", tag="fcp")[0:16, :]
                for kk in range(2):
                    nc.tensor.matmul(p3[:], fw3[kk][:], a2[kk][:],
                                     start=(kk == 0), stop=(kk == 1))
                o_sb = hp.tile([16, 1], F32)
                nc.scalar.activation(o_sb[:], p3[:], AF.Identity, bias=fb3[:], scale=1.0)
                nc.sync.dma_start(out, o_sb[:])

            if debug:
                nc.sync.dma_start(dbg["h3d"], h3[:])

    nc.finalize()
    return nc


def get_nc(debug=False):
    key = bool(debug)
    if key not in _NC_CACHE:
        _NC_CACHE[key] = _builder(debug=debug)
    return _NC_CACHE[key]


def make_in_maps(x, W1, b1, W2, b2, W3, b3, fc1_w, fc1_b, fc2_w, fc2_b, fc3_w, fc3_b):
    f32 = np.float32
    x = np.asarray(x, f32)
    B = x.shape[0]
    W1, W2, W3 = np.asarray(W1, f32), np.asarray(W2, f32), np.asarray(W3, f32)
    shared = {
        "idn": np.eye(128, dtype=f32),
        "AB1": np.concatenate([W1[:3] - W1[3:6], np.asarray(b1, f32)[None]], 0),
        "BB1": np.ascontiguousarray(W1[3:6]),
        "AB2": np.concatenate([W2[:64] - W2[64:], np.asarray(b2, f32)[None]], 0),
        "BB2": np.ascontiguousarray(W2[64:]),
        "AB3": np.ascontiguousarray(W3[:128] - W3[128:]),
        "BB3": np.ascontiguousarray(W3[128:]),
        "b3r": np.asarray(b3, f32)[None],
        "fc1w": np.asarray(fc1_w, f32),
        "fc1b": np.ascontiguousarray(np.asarray(fc1_b, f32).reshape(4, 128).T),
        "fc2w": np.asarray(fc2_w, f32),
        "fc2b": np.ascontiguousarray(np.asarray(fc2_b, f32).reshape(2, 128).T),
        "fc3w": np.pad(np.asarray(fc3_w, f32), ((0, 0), (0, 6))),
        "fc3b": np.pad(np.asarray(fc3_b, f32), (0, 6))[:, None],
    }
    in_maps = []
    for bb in range(B):
        xb = x[bb]
        m = dict(shared)
        m["xT"] = np.ascontiguousarray(xb.T)
        m["nsq1"] = -(xb * xb).sum(-1)[None, :].astype(f32)
        in_maps.append(m)
    return in_maps


def kernel(x, k, W1, b1, W2, b2, W3, b3, fc1_w, fc1_b, fc2_w, fc2_b, fc3_w, fc3_b,
           debug=False):
    from concourse import bass_utils
    x = np.asarray(x)
    assert int(k) == 20 and x.shape[1] == 2048 and x.shape[2] == 3
    B = x.shape[0]
    assert B == 8
    nc = get_nc(debug=debug)
    in_maps = make_in_maps(x, W1, b1, W2, b2, W3, b3,
                           fc1_w, fc1_b, fc2_w, fc2_b, fc3_w, fc3_b)
    res = bass_utils.run_bass_kernel_spmd(nc, in_maps, core_ids=list(range(B)))
    outs = np.stack([res.results[bb]["out"][:10, 0] for bb in range(B)], axis=0)
    if debug:
        return outs.astype(np.float32), res
    return outs.astype(np.float32)



# revision 9
# speedup vs baseline: 1.8313x; 1.8313x over previous
"""DGCNN (3x DynamicEdgeConv + global max pool + MLP head) on 8 Trainium2
NeuronCores, data-parallel over the batch (one point cloud per core).

EdgeConv algebra: h_ij = [x_i, x_j - x_i] @ W + b = u_i + v_j with
  u = x @ (Wa - Wb) + b,  v = x @ Wb;  out_i = u_i + max_{j in knn(i)} v_j.

kNN selection via quantized packed keys:
  key_ij = round(Relu(2S x_i.x_j + B)) + round(-S|x_j|^2) + (127 - j%128)/128
The integer part orders by quantized distance (Relu clamps only far pairs);
the fraction carries the chunk-local column index (reverse order so ties
prefer smaller j, like jax top_k). Top-24 per row: 16x max8 over 128-wide
chunks -> 128 candidates -> 3x(max8+max_index+match_replace). Indices are
recovered from fraction (j%128) + max_index position (chunk id) with no
full-width MaxIndex/MatchReplace passes.

Neighbor rows of v are fetched with ONE dma_gather per tile (2560 rows,
SWDGE cost 994ns fixed + 0.34ns/row) instead of 20 per-slot indirect DMAs;
the int16 index list is staged m-major + 16-partition-wrapped + replicated
x8 via small DMAs through DRAM. v is fp16 for layers 2/3 (fp32 layer 1:
dma_gather needs 256B-multiple rows).
"""
import numpy as np

_NC_CACHE = {}

N, NT, K = 2048, 16, 20
CH = 128          # selection chunk width
NCH = N // CH     # 16 chunks
NSEL = 24
SQ = (2048.0, 4096.0, 8192.0)      # dist quantization scale per layer
BQ = (16384.0, 16384.0, 32768.0)   # Relu clamp bias per layer


def _builder():
    import concourse.bacc as bacc
    import concourse.mybir as mybir
    from concourse.bass_types import AP
    from concourse import library_config
    from concourse.tile import TileContext

    F32 = mybir.dt.float32
    F16 = mybir.dt.float16
    I32 = mybir.dt.int32
    I16 = mybir.dt.int16
    U32 = mybir.dt.uint32
    AF = mybir.ActivationFunctionType
    ALU = mybir.AluOpType
    AX = mybir.AxisListType

    def ts(i, s):
        return slice(i * s, (i + 1) * s)

    nc = bacc.Bacc("TRN2", num_devices=8, dynamic_dma_scratch_size=49152)

    xT = nc.dram_tensor("xT", [3, N], F32, kind="ExternalInput").ap()
    comb1_in = nc.dram_tensor("comb1", [1, N], F32, kind="ExternalInput").ap()
    iot_in = nc.dram_tensor("iot", [1, N], F32, kind="ExternalInput").ap()
    idn_in = nc.dram_tensor("idn", [128, 128], F32, kind="ExternalInput").ap()
    AB1 = nc.dram_tensor("AB1", [4, 64], F32, kind="ExternalInput").ap()
    BB1 = nc.dram_tensor("BB1", [3, 64], F32, kind="ExternalInput").ap()
    AB2 = nc.dram_tensor("AB2", [65, 128], F32, kind="ExternalInput").ap()
    BB2 = nc.dram_tensor("BB2", [64, 128], F32, kind="ExternalInput").ap()
    AB3 = nc.dram_tensor("AB3", [128, 256], F32, kind="ExternalInput").ap()
    BB3 = nc.dram_tensor("BB3", [128, 256], F32, kind="ExternalInput").ap()
    b3r = nc.dram_tensor("b3r", [1, 256], F32, kind="ExternalInput").ap()
    fc1w = nc.dram_tensor("fc1w", [256, 512], F32, kind="ExternalInput").ap()
    fc1b = nc.dram_tensor("fc1b", [128, 4], F32, kind="ExternalInput").ap()
    fc2w = nc.dram_tensor("fc2w", [512, 256], F32, kind="ExternalInput").ap()
    fc2b = nc.dram_tensor("fc2b", [128, 2], F32, kind="ExternalInput").ap()
    fc3w = nc.dram_tensor("fc3w", [256, 16], F32, kind="ExternalInput").ap()
    fc3b = nc.dram_tensor("fc3b", [16, 1], F32, kind="ExternalInput").ap()
    out = nc.dram_tensor("out", [16, 1], F32, kind="ExternalOutput").ap()

    with TileContext(nc) as tc:
        with tc.tile_pool(name="const", bufs=1) as cp, \
             tc.tile_pool(name="feat", bufs=1) as fp, \
             tc.tile_pool(name="vdram", bufs=1, space="DRAM") as vdp:
            nc.gpsimd.load_library(library_config.mlp)

            v_drams = {
                1: vdp.tile([N, 64], F32, name="v_dram1"),
                2: vdp.tile([N, 128], F16, name="v_dram2"),
                3: vdp.tile([N, 256], F16, name="v_dram3"),
            }
            idn = cp.tile([128, 128], F32)
            nc.sync.dma_start(idn[:], idn_in)
            ones1 = cp.tile([1, 128], F32)
            nc.vector.memset(ones1[:], 1.0)
            iot = cp.tile([1, N], F32)
            nc.sync.dma_start(iot[:], iot_in)
            zb1 = cp.tile([1, 1], F32)
            nc.vector.memset(zb1[:], 0.0)
            with tc.tile_pool(name="warm", bufs=1, space="PSUM") as wps:
                wt = wps.tile([128, 128], F32)
                nc.tensor.transpose(wt[:], idn[:], idn[:])

            hT1 = fp.tile([4, N], F32)
            crow1 = fp.tile([1, N], F32)
            hT2 = fp.tile([65, N], F32)
            crow2 = fp.tile([1, N], F32)
            hT3 = fp.tile([128, N], F32)
            crow3 = fp.tile([1, N], F32)
            h3 = fp.tile([128, NT, 256], F32)

            nc.vector.memset(hT1[:], 1.0)
            nc.sync.dma_start(hT1[0:3, :], xT)
            nc.sync.dma_start(crow1[:], comb1_in)

            with tc.tile_pool(name="wts", bufs=1) as wp:
                w = {}
                for nm, ap_, shape in [("AB1", AB1, [4, 64]), ("BB1", BB1, [3, 64]),
                                       ("AB2", AB2, [65, 128]), ("BB2", BB2, [64, 128]),
                                       ("AB3", AB3, [128, 256]), ("BB3", BB3, [128, 256]),
                                       ("b3r", b3r, [1, 256])]:
                    t = wp.tile(shape, F32, name=f"w_{nm}")
                    nc.sync.dma_start(t[:], ap_)
                    w[nm] = t

                def run_layer(layer, C, D, hT, crow, Acat, Bw, brow, out_h, hT_next,
                              crow_next, next_C, S_next):
                    """One EdgeConv layer. out_h: [128, NT, D] f32 output tile.
                    If hT_next is not None, also emits transposes + next-layer
                    nsq/comb rows."""
                    S, B = SQ[layer - 1], BQ[layer - 1]
                    VD = F32 if layer == 1 else F16
                    u_aug = C + 1 <= 128
                    vslice = v_drams[layer]
                    with tc.tile_pool(name=f"L{layer}", bufs=1) as lp:
                        hT2x = lp.tile([C, N], F32, name="hT2x")
                        nc.scalar.activation(hT2x[:], hT[0:C, :], AF.Copy,
                                             scale=2.0 * S)
                        comb = lp.tile([128, N], F32, name="comb")
                        nc.gpsimd.partition_broadcast(comb[:], crow[:])
                        biasB = lp.tile([128, 1], F32, name="biasB")
                        nc.vector.memset(biasB[:], B)
                        u = lp.tile([128, NT, D], F32, name="u")
                        if out_h is None:
                            out_h = lp.tile([128, NT, D], F32, name="hout")

                        with tc.tile_pool(name=f"L{layer}uvp", bufs=2, space="PSUM") as uvps, \
                             tc.tile_pool(name=f"L{layer}uvs", bufs=4) as uvsb, \
                             tc.tile_pool(name=f"L{layer}dp", bufs=2, space="PSUM") as dps, \
                             tc.tile_pool(name=f"L{layer}sel", bufs=2) as selp, \
                             tc.tile_pool(name=f"L{layer}sm", bufs=3) as smp, \
                             tc.tile_pool(name=f"L{layer}dr", bufs=3, space="DRAM") as drp, \
                             tc.tile_pool(name=f"L{layer}g", bufs=2) as gp:
                            # ---------- phase A: u/v matmuls, v -> DRAM ----------
                            for t in range(NT):
                                vp = uvps.tile([128, D], F32, name="vp")
                                nc.tensor.matmul(vp[:], hT[0:C, ts(t, 128)], Bw[:],
                                                 start=True, stop=True)
                                up = uvps.tile([128, D], F32, name="up")
                                if u_aug:
                                    nc.tensor.matmul(up[:], hT[0:C + 1, ts(t, 128)],
                                                     Acat[:], start=True, stop=True)
                                else:
                                    nc.tensor.matmul(up[:], hT[0:C, ts(t, 128)],
                                                     Acat[:], start=True, stop=False)
                                    nc.tensor.matmul(up[:], ones1[:], brow[:],
                                                     start=False, stop=True)
                                vsb = uvsb.tile([128, D], VD, name="vsb")
                                nc.scalar.copy(vsb[:], vp[:])
                                nc.scalar.copy(u[:, t, :], up[:])
                                nc.sync.dma_start(vslice[ts(t, 128), :], vsb[:])

                            # ---------- phase B: dist, select, gather ----------
                            for t in range(NT):
                                ddq = selp.tile([128, N], I32, name="ddq")
                                for hh in range(2):
                                    dp = dps.tile([128, 1024], F32, name="dp")
                                    for jc in range(2):
                                        nc.tensor.matmul(
                                            dp[:, ts(jc, 512)],
                                            hT2x[:, ts(t, 128)],
                                            hT[0:C, ts(hh * 2 + jc, 512)],
                                            start=True, stop=True)
                                    nc.scalar.activation(ddq[:, ts(hh, 1024)], dp[:],
                                                         AF.Relu, bias=biasB[:],
                                                         scale=1.0)
                                packed = ddq[:].bitcast(F32)
                                nc.vector.tensor_tensor(out=packed, in0=ddq[:],
                                                        in1=comb[:], op=ALU.add)

                                cand = smp.tile([128, NCH * 8], F32, name="cand")
                                for c in range(NCH):
                                    nc.vector.max(out=cand[:, ts(c, 8)],
                                                  in_=packed[:, ts(c, CH)])  # noqa
                                mcat = smp.tile([128, NSEL], F32, name="mcat")
                                cidx = smp.tile([128, NSEL], U32, name="cidx")
                                ww = smp.tile([128, NCH * 8], F32, name="ww")
                                nc.vector.max(out=mcat[:, 0:8], in_=cand[:])
                                nc.vector.max_index(out=cidx[:, 0:8],
                                                    in_max=mcat[:, 0:8], in_values=cand[:])
                                nc.vector.match_replace(out=ww[:], in_to_replace=mcat[:, 0:8],
                                                        in_values=cand[:], imm_value=-1e30)
                                nc.vector.max(out=mcat[:, 8:16], in_=ww[:])
                                nc.vector.max_index(out=cidx[:, 8:16],
                                                    in_max=mcat[:, 8:16], in_values=ww[:])
                                nc.vector.match_replace(out=ww[:], in_to_replace=mcat[:, 8:16],
                                                        in_values=ww[:], imm_value=-1e30)
                                nc.vector.max(out=mcat[:, 16:24], in_=ww[:])
                                nc.vector.max_index(out=cidx[:, 16:24],
                                                    in_max=mcat[:, 16:24], in_values=ww[:])

                                # index extraction on slots 1..20 (slot 0 = self).
                                # m*CH = CH*I + (CH-1 - j%CH) exactly (< 2^24), so
                                # int-cast (round OR trunc, value is integer) then
                                # &(CH-1) recovers the reversed local index; j =
                                # chunk*CH + CH-1 - rev = cb' - rev.
                                mm20 = mcat[:, 1:21]
                                ci20 = cidx[:, 1:21]
                                t1 = smp.tile([128, K], F32, name="t1")
                                nc.vector.tensor_scalar(out=t1[:], in0=mm20,
                                                        scalar1=float(CH),
                                                        scalar2=None, op0=ALU.mult)
                                ii = smp.tile([128, K], I32, name="ii")
                                nc.vector.tensor_copy(ii[:], t1[:])
                                rv = smp.tile([128, K], I32, name="rv")
                                nc.vector.tensor_scalar(out=rv[:], in0=ii[:],
                                                        scalar1=CH - 1, scalar2=None,
                                                        op0=ALU.bitwise_and)
                                sh = smp.tile([128, K], U32, name="sh")
                                nc.vector.tensor_scalar(out=sh[:], in0=ci20, scalar1=3,
                                                        scalar2=None,
                                                        op0=ALU.logical_shift_right)
                                cb = smp.tile([128, K], F32, name="cb")
                                nc.vector.tensor_scalar(out=cb[:], in0=sh[:],
                                                        scalar1=float(CH),
                                                        scalar2=float(CH - 1),
                                                        op0=ALU.mult, op1=ALU.add)
                                jf = smp.tile([128, K], F32, name="jf")
                                nc.vector.tensor_tensor(out=jf[:], in0=cb[:], in1=rv[:],
                                                        op=ALU.subtract)
                                jt16 = smp.tile([128, K], I16, name="jt16")
                                nc.vector.tensor_copy(jt16[:], jf[:])

                                # wrap to m-major 16-partition-replicated layout
                                scrA = drp.tile([128, K], I16, name="scrA")
                                nc.sync.dma_start(scrA[:], jt16[:])
                                w16 = smp.tile([16, K * 8], I16, name="w16")
                                sా = scrA[:]
                                nc.sync.dma_start(w16[:], AP(
                                    tensor=sా.tensor, offset=sా.offset,
                                    ap=[[K, 16], [1, K], [16 * K, 8]]))
                                scrB = drp.tile([16, K * 8], I16, name="scrB")
                                nc.scalar.dma_start(scrB[:], w16[:])
                                wrep = smp.tile([128, K * 8], I16, name="wrep")
                                sbp = scrB[:]
                                nc.scalar.dma_start(wrep[:], AP(
                                    tensor=sbp.tensor, offset=sbp.offset,
                                    ap=[[0, 8], [K * 8, 16], [1, K * 8]]))

                                # dma_gather ucode caps at 1024 indices/call
                                gb = gp.tile([128, K, D], VD, name="gb")
                                for m0, m1 in ((0, 8), (8, 16), (16, 20)):
                                    nidx = (m1 - m0) * 128
                                    nc.gpsimd.dma_gather(
                                        out_ap=gb[:, m0:m1, :], in_ap=vslice[:],
                                        idxs_ap=wrep[:, m0 * 8:m1 * 8],
                                        num_idxs=nidx, num_idxs_reg=nidx,
                                        elem_size=D)

                                vm = gp.tile([128, D], F32, name="vm")
                                if layer == 1:
                                    nc.vector.tensor_reduce(
                                        out=vm[:], in_=gb[:].rearrange("p m d -> p d m"),
                                        axis=AX.X, op=ALU.max)
                                else:
                                    p10 = gp.tile([128, 10, D], VD, name="p10")
                                    nc.vector.tensor_tensor(out=p10[:], in0=gb[:, 0:10, :],
                                                            in1=gb[:, 10:20, :], op=ALU.max)
                                    p5 = gp.tile([128, 5, D], VD, name="p5")
                                    nc.vector.tensor_tensor(out=p5[:], in0=p10[:, 0:5, :],
                                                            in1=p10[:, 5:10, :], op=ALU.max)
                                    p2 = gp.tile([128, 2, D], VD, name="p2")
                                    nc.vector.tensor_tensor(out=p2[:], in0=p5[:, 0:2, :],
                                                            in1=p5[:, 2:4, :], op=ALU.max)
                                    p1 = gp.tile([128, 1, D], VD, name="p1")
                                    nc.vector.tensor_tensor(out=p1[:], in0=p2[:, 0:1, :],
                                                            in1=p2[:, 1:2, :], op=ALU.max)
                                    nc.vector.tensor_tensor(out=vm[:], in0=p1[:, 0, :],
                                                            in1=p5[:, 4, :], op=ALU.max)
                                nc.vector.tensor_tensor(out=out_h[:, t, :], in0=u[:, t, :],
                                                        in1=vm[:], op=ALU.add)

                        # ---------- phase C: transposes + next-layer rows ----------
                        if hT_next is not None:
                            with tc.tile_pool(name=f"L{layer}tp", bufs=3, space="PSUM") as tps, \
                                 tc.tile_pool(name=f"L{layer}ts", bufs=2) as tsp:
                                for t in range(NT):
                                    tp = tps.tile([next_C, 128], F32, name="tp")
                                    nc.tensor.transpose(tp[:], out_h[:, t, 0:next_C], idn[:])
                                    nc.scalar.copy(hT_next[0:next_C, ts(t, 128)], tp[:])
                                if next_C + 1 <= hT_next.shape[0]:
                                    nc.vector.memset(hT_next[next_C:next_C + 1, :], 1.0)
                                xsq = tsp.tile([next_C, N], F32, name="xsq")
                                nc.scalar.square(xsq[:], hT_next[0:next_C, :])
                                onesC = tsp.tile([next_C, 1], F32, name="onesC")
                                nc.vector.memset(onesC[:], 1.0)
                                ni32 = tsp.tile([1, N], I32, name="ni32")
                                for jc in range(4):
                                    sqp = tps.tile([1, 512], F32, name="sqp")
                                    nc.tensor.matmul(sqp[:], onesC[:], xsq[:, ts(jc, 512)],
                                                     start=True, stop=True)
                                    nc.scalar.activation(ni32[:, ts(jc, 512)], sqp[:],
                                                         AF.Copy, scale=-S_next)
                                nc.vector.tensor_tensor(out=crow_next[:], in0=ni32[:],
                                                        in1=iot[:], op=ALU.add)

                run_layer(1, 3, 64, hT1, crow1, w["AB1"], w["BB1"], None, None,
                          hT2, crow2, 64, SQ[1])
                run_layer(2, 64, 128, hT2, crow2, w["AB2"], w["BB2"], None, None,
                          hT3, crow3, 128, SQ[2])
                run_layer(3, 128, 256, hT3, crow3, w["AB3"], w["BB3"], w["b3r"], h3,
                          None, None, None, None)

            # ---------- head: global max pool + 3 FC layers ----------
            with tc.tile_pool(name="head", bufs=1) as hp, \
                 tc.tile_pool(name="headps", bufs=1, space="PSUM") as hps:
                gmax = hp.tile([128, 256], F32)
                nc.vector.tensor_reduce(out=gmax[:],
                                        in_=h3[:].rearrange("p g d -> p d g"),
                                        axis=AX.X, op=ALU.max)
                g0 = hp.tile([128, 1], F32)
                g1 = hp.tile([128, 1], F32)
                for half, gdst in ((0, g0), (1, g1)):
                    tp = hps.tile([128, 128], F32, name="tp", tag="tp")
                    nc.tensor.transpose(tp[:], gmax[:, ts(half, 128)], idn[:])
                    tsb = hp.tile([128, 128], F32, name=f"tsb_{half}")
                    nc.scalar.copy(tsb[:], tp[:])
                    nc.vector.tensor_reduce(out=gdst[:], in_=tsb[:], axis=AX.X, op=ALU.max)

                fw1 = [hp.tile([128, 512], F32, name=f"fw1_{kk}") for kk in range(2)]
                fw2 = [hp.tile([128, 256], F32, name=f"fw2_{kk}") for kk in range(4)]
                fw3 = [hp.tile([128, 16], F32, name=f"fw3_{kk}") for kk in range(2)]
                fb1 = hp.tile([128, 4], F32)
                fb2 = hp.tile([128, 2], F32)
                fb3 = hp.tile([16, 1], F32)
                for kk in range(2):
                    nc.sync.dma_start(fw1[kk][:], fc1w[ts(kk, 128), :])
                    nc.sync.dma_start(fw3[kk][:], fc3w[ts(kk, 128), :])
                for kk in range(4):
                    nc.sync.dma_start(fw2[kk][:], fc2w[ts(kk, 128), :])
                nc.sync.dma_start(fb1[:], fc1b)
                nc.sync.dma_start(fb2[:], fc2b)
                nc.sync.dma_start(fb3[:], fc3b)

                a1 = [hp.tile([128, 1], F32, name=f"a1_{m}") for m in range(4)]
                for m in range(4):
                    p = hps.tile([128, 1], F32, name="fcp", tag="fcp")
                    nc.tensor.matmul(p[:], fw1[0][:, ts(m, 128)], g0[:], start=True, stop=False)
                    nc.tensor.matmul(p[:], fw1[1][:, ts(m, 128)], g1[:], start=False, stop=True)
                    nc.scalar.activation(a1[m][:], p[:], AF.Relu, bias=fb1[:, m:m + 1], scale=1.0)
                a2 = [hp.tile([128, 1], F32, name=f"a2_{m}") for m in range(2)]
                for m in range(2):
                    p = hps.tile([128, 1], F32, name="fcp", tag="fcp")
                    for kk in range(4):
                        nc.tensor.matmul(p[:], fw2[kk][:, ts(m, 128)], a1[kk][:],
                                         start=(kk == 0), stop=(kk == 3))
                    nc.scalar.activation(a2[m][:], p[:], AF.Relu, bias=fb2[:, m:m + 1], scale=1.0)
                p3 = hps.tile([128, 1], F32, name="fcp", tag="fcp")[0:16, :]
                for kk in range(2):
                    nc.tensor.matmul(p3[:], fw3[kk][:], a2[kk][:],
                                     start=(kk == 0), stop=(kk == 1))
                o_sb = hp.tile([16, 1], F32)
                nc.scalar.activation(o_sb[:], p3[:], AF.Identity, bias=fb3[:], scale=1.0)
                nc.sync.dma_start(out, o_sb[:])

    nc.finalize()
    return nc


def get_nc(debug=False):
    if "nc" not in _NC_CACHE:
        _NC_CACHE["nc"] = _builder()
    return _NC_CACHE["nc"]


def make_in_maps(x, W1, b1, W2, b2, W3, b3, fc1_w, fc1_b, fc2_w, fc2_b, fc3_w, fc3_b):
    f32 = np.float32
    x = np.asarray(x, f32)
    B = x.shape[0]
    W1, W2, W3 = np.asarray(W1, f32), np.asarray(W2, f32), np.asarray(W3, f32)
    j = np.arange(N)
    iotafrac = ((CH - 1 - (j % CH)) / float(CH)).astype(f32)[None, :]
    shared = {
        "idn": np.eye(128, dtype=f32),
        "iot": iotafrac,
        "AB1": np.concatenate([W1[:3] - W1[3:6], np.asarray(b1, f32)[None]], 0),
        "BB1": np.ascontiguousarray(W1[3:6]),
        "AB2": np.concatenate([W2[:64] - W2[64:], np.asarray(b2, f32)[None]], 0),
        "BB2": np.ascontiguousarray(W2[64:]),
        "AB3": np.ascontiguousarray(W3[:128] - W3[128:]),
        "BB3": np.ascontiguousarray(W3[128:]),
        "b3r": np.asarray(b3, f32)[None],
        "fc1w": np.asarray(fc1_w, f32),
        "fc1b": np.ascontiguousarray(np.asarray(fc1_b, f32).reshape(4, 128).T),
        "fc2w": np.asarray(fc2_w, f32),
        "fc2b": np.ascontiguousarray(np.asarray(fc2_b, f32).reshape(2, 128).T),
        "fc3w": np.pad(np.asarray(fc3_w, f32), ((0, 0), (0, 6))),
        "fc3b": np.pad(np.asarray(fc3_b, f32), (0, 6))[:, None],
    }
    in_maps = []
    for bb in range(B):
        xb = x[bb]
        m = dict(shared)
        m["xT"] = np.ascontiguousarray(xb.T)
        nsq = (xb * xb).sum(-1).astype(np.float64)
        m["comb1"] = (np.rint(-SQ[0] * nsq) + iotafrac[0].astype(np.float64)
                      ).astype(f32)[None, :]
        in_maps.append(m)
    return in_maps


def kernel(x, k, W1, b1, W2, b2, W3, b3, fc1_w, fc1_b, fc2_w, fc2_b, fc3_w, fc3_b):
    from concourse import bass_utils
    x = np.asarray(x)
    assert int(k) == 20 and x.shape[1] == N and x.shape[2] == 3
    B = x.shape[0]
    assert B == 8
    nc = get_nc()
    in_maps = make_in_maps(x, W1, b1, W2, b2, W3, b3,
                           fc1_w, fc1_b, fc2_w, fc2_b, fc3_w, fc3_b)
    res = bass_utils.run_bass_kernel_spmd(nc, in_maps, core_ids=list(range(B)))
    outs = np.stack([res.results[bb]["out"][:10, 0] for bb in range(B)], axis=0)
    return outs.astype(np.float32)
